# revision 16
# baseline (speedup 1.0000x reference)
"""Trainium2 Bass kernel for ExcitationEmbedding + Ion RoPE.

Computes, for inputs
  excitations [256, 512, 2] int64 (pairs (a, b) with a, b in [0, 6)),
  n_electrons [256] f32, n_protons [256] f32,
  emb_weight  [26, 256] f32, lookup_table [6, 6] int64:

  idx   = lookup_table[a, b]                       # [B, N]
  emb   = emb_weight[idx]                          # [B, N, D]
  out   = per-batch block-diagonal rotation of emb (theta from n_electrons,
          phi from n_protons, 4-wide blocks: dims (0,1) by theta, (2,3) by phi)

Strategy (pure data parallel over 8 cores, 32 batches each):
  - flat code f = 6*a + b in [0, 36); one-hot against an iota constant
    (single fused is_equal over all batches, fp16).
  - emb36[j] = emb_weight[lut[j]] built once with a select-matmul (invalid
    j rows are 0); rotated tables for ALL batches are built with 3 big
    [36, 32*256] fp16 DVE ops, with the per-batch cos/sin patterns
    replicated across the 36 table partitions via gpsimd partition
    broadcasts (sources read back from DRAM as single contiguous rows).
  - Gather is a PE fp16 matmul: chunk c of batch b covers tokens {4k+c},
    so each PSUM evacuation lands 4 consecutive tokens per partition and
    the per-batch 512 KB output DMA is fully contiguous.
"""

import functools

import numpy as np

import concourse.bass as bass
import concourse.bacc as bacc
import concourse.mybir as mybir
from concourse import tile
from concourse.bass_utils import run_bass_kernel_spmd

B, N, D = 256, 512, 256
N_CORES = 8
BL = B // N_CORES  # 32 batches per core
ANGLE_SCALE = 0.05
HALF_PI = float(np.pi / 2)

F32 = mybir.dt.float32
F16 = mybir.dt.float16
I32 = mybir.dt.int32
AF = mybir.ActivationFunctionType
ALU = mybir.AluOpType

# evacuation-engine split: chunk c of batch b goes to DVE if EVAC_DVE[b][c]
EVAC_DVE = [[c == 0 or (c == 1 and b < 20) for c in range(4)] for b in range(BL)]


def build_bass() -> bass.Bass:
    nc = bacc.Bacc(
        "TRN2", target_bir_lowering=False, debug=False, num_devices=N_CORES
    )

    exc = nc.dram_tensor("exc", [BL, N * 2], I32, kind="ExternalInput")
    ne = nc.dram_tensor("ne", [BL, 1], F32, kind="ExternalInput")
    npr = nc.dram_tensor("npr", [BL, 1], F32, kind="ExternalInput")
    emb = nc.dram_tensor("emb", [26, D], F32, kind="ExternalInput")
    lut = nc.dram_tensor("lut", [1, 36], F32, kind="ExternalInput")
    out = nc.dram_tensor("out", [BL, N, D], F32, kind="ExternalOutput")

    iota_f32 = nc.inline_tensor(
        np.arange(36, dtype=np.float32).reshape(36, 1), "iota_f32")

    with tile.TileContext(nc) as tc:
        with (
            tc.tile_pool(name="const", bufs=1) as const,
            tc.tile_pool(name="opool", bufs=3) as opool,
            tc.tile_pool(name="dram", bufs=1, space="DRAM") as dram,
            tc.tile_pool(name="psum_s", bufs=1, space="PSUM") as psum_s,
            tc.tile_pool(name="psum", bufs=6, space="PSUM") as psum,
        ):
            # ---- loads ----
            exc_s = const.tile([BL, N * 2], I32)
            nc.sync.dma_start(out=exc_s[:], in_=exc[:])
            ne_s = const.tile([BL, 1], F32)
            nc.sync.dma_start(out=ne_s[:], in_=ne[:])
            npr_s = const.tile([BL, 1], F32)
            nc.sync.dma_start(out=npr_s[:], in_=npr[:])
            emb_s = const.tile([26, D], F16)
            nc.gpsimd.dma_start(out=emb_s[:], in_=emb[:])  # casts f32->f16
            iota_s = const.tile([36, 1], F32)
            nc.sync.dma_start(out=iota_s[:], in_=iota_f32[:])

            # ---- per-batch angle columns [BL, 1] ----
            hp = const.tile([BL, 1], F32)
            nc.vector.memset(hp[:], HALF_PI)
            # cos(t) = sin(pi/2 - t) keeps the LUT argument within [-pi, pi]
            ct = const.tile([BL, 1], F32)
            nc.scalar.activation(ct[:], ne_s[:], AF.Sin, bias=hp[:],
                                 scale=-ANGLE_SCALE)
            st = const.tile([BL, 1], F32)
            nc.scalar.activation(st[:], ne_s[:], AF.Sin, bias=0.0, scale=ANGLE_SCALE)
            nst = const.tile([BL, 1], F32)
            nc.scalar.activation(nst[:], ne_s[:], AF.Sin, bias=0.0, scale=-ANGLE_SCALE)
            cp = const.tile([BL, 1], F32)
            nc.scalar.activation(cp[:], npr_s[:], AF.Sin, bias=hp[:],
                                 scale=-ANGLE_SCALE)
            sp = const.tile([BL, 1], F32)
            nc.scalar.activation(sp[:], npr_s[:], AF.Sin, bias=0.0, scale=ANGLE_SCALE)
            nsp = const.tile([BL, 1], F32)
            nc.scalar.activation(nsp[:], npr_s[:], AF.Sin, bias=0.0, scale=-ANGLE_SCALE)

            # natural layout: C_all[b, 4k+i] = (ct,ct,cp,cp)[i],
            #                 S_all[b, 4k+i] = (st,-st,sp,-sp)[i]
            ones = const.tile([BL, 64], F16)
            nc.vector.memset(ones[:], 1.0)
            c_all = const.tile([BL, D], F16)
            s_all = const.tile([BL, D], F16)
            c4 = c_all[:].rearrange("q (k i) -> q k i", i=4)
            s4 = s_all[:].rearrange("q (k i) -> q k i", i=4)
            for i, col in enumerate([ct, ct, cp, cp]):
                nc.vector.tensor_scalar(out=c4[:, :, i], in0=ones[:],
                                        scalar1=col[:], scalar2=None, op0=ALU.mult)
            for i, col in enumerate([st, nst, sp, nsp]):
                nc.vector.tensor_scalar(out=s4[:, :, i], in0=ones[:],
                                        scalar1=col[:], scalar2=None, op0=ALU.mult)

            # ---- select matrix: selT[r, j] = (lut_flat[j] == r), r in [0,26) ----
            lut_bc = const.tile([26, 36], F32)
            nc.sync.dma_start(out=lut_bc[:], in_=lut[0:1, :].to_broadcast((26, 36)))
            selT = const.tile([26, 36], F16)
            nc.vector.tensor_scalar(out=selT[:], in0=lut_bc[:],
                                    scalar1=iota_s[0:26, :], scalar2=None,
                                    op0=ALU.is_equal)

            # ---- 36-row gathered table (natural layout) + pair-swapped copy
            eph_ps = psum_s.tile([36, D], F32)
            nc.tensor.matmul(eph_ps[:], selT[:], emb_s[:], start=True, stop=True)
            e_ph = const.tile([36, D], F16)
            nc.scalar.activation(e_ph[:], eph_ps[:], AF.Copy)
            e_sw = const.tile([36, D], F16)
            ep2 = e_ph[:].rearrange("j (k i) -> j k i", i=2)
            es2 = e_sw[:].rearrange("j (k i) -> j k i", i=2)
            nc.vector.tensor_copy(es2[:, :, 0], ep2[:, :, 1])
            nc.vector.tensor_copy(es2[:, :, 1], ep2[:, :, 0])

            # ---- flat codes: flat[b, n] = 6*a + b (fp16, values < 36) ----
            exc3 = exc_s[:].rearrange("q (n two) -> q n two", two=2)
            a_f = const.tile([BL, N], F32)
            nc.vector.tensor_copy(a_f[:], exc3[:, :, 0])
            b_f = const.tile([BL, N], F32)
            nc.vector.tensor_copy(b_f[:], exc3[:, :, 1])
            flat = const.tile([BL, N], F16)
            nc.vector.scalar_tensor_tensor(out=flat[:], in0=a_f[:], scalar=6.0,
                                           in1=b_f[:], op0=ALU.mult, op1=ALU.add)

            # ---- DRAM bounce -> single-row reads -> partition broadcasts ----
            flat_d = dram.tile([BL, N], F16)
            nc.sync.dma_start(out=flat_d[:], in_=flat[:])
            c_all_d = dram.tile([BL, D], F16)
            nc.sync.dma_start(out=c_all_d[:], in_=c_all[:])
            s_all_d = dram.tile([BL, D], F16)
            nc.sync.dma_start(out=s_all_d[:], in_=s_all[:])

            flat_row = const.tile([1, BL * N], F16)
            nc.sync.dma_start(
                out=flat_row[:],
                in_=flat_d[:].rearrange("q n -> (q n)").unsqueeze(0))
            c_row = const.tile([1, BL * D], F16)
            nc.sync.dma_start(
                out=c_row[:],
                in_=c_all_d[:].rearrange("q d -> (q d)").unsqueeze(0))
            s_row = const.tile([1, BL * D], F16)
            nc.sync.dma_start(
                out=s_row[:],
                in_=s_all_d[:].rearrange("q d -> (q d)").unsqueeze(0))

            flat_big = const.tile([36, BL, N], F16)
            nc.gpsimd.partition_broadcast(flat_big[:], flat_row[:])
            cb = const.tile([36, BL, D], F16)
            nc.gpsimd.partition_broadcast(cb[:], c_row[:])
            sb = const.tile([36, BL, D], F16)
            nc.gpsimd.partition_broadcast(sb[:], s_row[:])

            # ---- one-hot for all batches (single op, in place) ----
            nc.vector.tensor_scalar(out=flat_big[:], in0=flat_big[:],
                                    scalar1=iota_s[:], scalar2=None,
                                    op0=ALU.is_equal)
            onehot = flat_big

            # ---- rotated tables for all batches: rot = e_ph*cb + e_sw*sb ----
            eph_b = e_ph[:].unsqueeze(1).broadcast_to((36, BL, D))
            esw_b = e_sw[:].unsqueeze(1).broadcast_to((36, BL, D))
            t1 = const.tile([36, BL, D], F16)
            nc.vector.tensor_mul(t1[:], eph_b, cb[:])
            t2 = const.tile([36, BL, D], F16)
            nc.vector.tensor_mul(t2[:], esw_b, sb[:])
            nc.vector.tensor_add(t1[:], t1[:], t2[:])
            rot = t1

            # ---- gather matmuls + evacuation + output DMA ----
            for b in range(BL):
                obuf = opool.tile([128, 4 * D], F32)
                for c in range(4):
                    ps = psum.tile([128, D], F32)
                    # chunk c covers tokens {4k + c}: stride-4 weight columns
                    nc.tensor.matmul(ps[:], onehot[:, b, c::4], rot[:, b, :],
                                     start=True, stop=True)
                    dst = obuf[:, c * D:(c + 1) * D]
                    if EVAC_DVE[b][c]:
                        nc.vector.tensor_copy(dst, ps[:])
                    else:
                        nc.scalar.activation(dst, ps[:], AF.Copy)

                # token t = 4k + c lives at obuf[k, c*256:(c+1)*256] -> the
                # DRAM view below is fully linear (contiguous 512 KB write)
                nc.sync.dma_start(
                    out=out[b].rearrange("(p c) d -> p c d", p=128),
                    in_=obuf[:])

    nc.compile()
    return nc


@functools.lru_cache(maxsize=1)
def _get_nc() -> bass.Bass:
    return build_bass()


def kernel_with_results(excitations, n_electrons, n_protons, emb_weight,
                        lookup_table, trace=False):
    exc = np.ascontiguousarray(np.asarray(excitations)).astype(np.int64)
    exc32 = exc.astype(np.int32).reshape(B, N * 2)
    ne = np.ascontiguousarray(np.asarray(n_electrons, dtype=np.float32))
    npr = np.ascontiguousarray(np.asarray(n_protons, dtype=np.float32))
    emb = np.ascontiguousarray(np.asarray(emb_weight, dtype=np.float32))
    lut_f = np.asarray(lookup_table).astype(np.float32).reshape(1, 36)
    lut_f = np.ascontiguousarray(lut_f)

    in_maps = []
    for c in range(N_CORES):
        sl = slice(c * BL, (c + 1) * BL)
        in_maps.append({
            "exc": np.ascontiguousarray(exc32[sl]),
            "ne": np.ascontiguousarray(ne[sl].reshape(BL, 1)),
            "npr": np.ascontiguousarray(npr[sl].reshape(BL, 1)),
            "emb": emb,
            "lut": lut_f,
        })

    nc = _get_nc()
    res = run_bass_kernel_spmd(nc, in_maps, list(range(N_CORES)), trace=trace)
    out_arr = np.concatenate(
        [res.results[c]["out"] for c in range(N_CORES)], axis=0)
    return np.ascontiguousarray(out_arr.reshape(B, N, D).astype(np.float32)), res


def kernel(excitations, n_electrons, n_protons, emb_weight, lookup_table):
    out_arr, _ = kernel_with_results(excitations, n_electrons, n_protons,
                                     emb_weight, lookup_table)
    return out_arr


# revision 17
# speedup vs baseline: 1.3641x; 1.3641x over previous
"""Trainium2 Bass kernel for ExcitationEmbedding + Ion RoPE.

Computes, for inputs
  excitations [256, 512, 2] int64 (pairs (a, b) with a, b in [0, 6)),
  n_electrons [256] f32, n_protons [256] f32,
  emb_weight  [26, 256] f32, lookup_table [6, 6] int64:

  idx   = lookup_table[a, b]                       # [B, N]
  emb   = emb_weight[idx]                          # [B, N, D]
  out   = per-batch block-diagonal rotation of emb (theta from n_electrons,
          phi from n_protons, 4-wide blocks: dims (0,1) by theta, (2,3) by phi)

Strategy (pure data parallel over 8 cores, 32 batches each):
  - flat code f = 6*a + b in [0, 36); one-hot against an iota constant.
  - emb36[j] = emb_weight[lut[j]] built once with a select-matmul (invalid
    j rows are 0). Per-batch rotated tables rot[j, b, d] are built with
    plain fp16 tensor ops on group tiles; the per-batch cos/sin patterns
    and flat codes are replicated across the 36 table partitions via
    DRAM->DRAM row staging + contiguous read-back (big DMA packets).
  - Gather is a PE fp16 matmul: chunk c of batch b covers tokens {4k+c};
    all 4 chunks land in one 2-bank PSUM tile, evacuated with a single
    [128, 1024] copy, and the per-batch 512 KB output DMA is fully linear.
  - Work is pipelined in 8 groups of 4 batches.
"""

import functools

import numpy as np

import concourse.bass as bass
import concourse.bacc as bacc
import concourse.mybir as mybir
from concourse import tile
from concourse.bass_utils import run_bass_kernel_spmd

B, N, D = 256, 512, 256
N_CORES = 8
BL = B // N_CORES   # 32 batches per core
G = 4               # batches per pipeline group
ANGLE_SCALE = 0.05
HALF_PI = float(np.pi / 2)

F32 = mybir.dt.float32
F16 = mybir.dt.float16
I16 = mybir.dt.int16
AF = mybir.ActivationFunctionType
ALU = mybir.AluOpType


def build_bass() -> bass.Bass:
    nc = bacc.Bacc(
        "TRN2", target_bir_lowering=False, debug=False, num_devices=N_CORES
    )

    exc = nc.dram_tensor("exc", [BL, N * 2], I16, kind="ExternalInput")
    ne = nc.dram_tensor("ne", [BL, 1], F32, kind="ExternalInput")
    npr = nc.dram_tensor("npr", [BL, 1], F32, kind="ExternalInput")
    emb = nc.dram_tensor("emb", [26, D], F32, kind="ExternalInput")
    lut = nc.dram_tensor("lut", [1, 36], F32, kind="ExternalInput")
    out = nc.dram_tensor("out", [BL, N, D], F32, kind="ExternalOutput")

    iota_f32 = nc.inline_tensor(
        np.arange(36, dtype=np.float32).reshape(36, 1), "iota_f32")

    with tile.TileContext(nc) as tc:
        with (
            tc.tile_pool(name="const", bufs=1) as const,
            tc.tile_pool(name="gpool", bufs=3) as gpool,
            tc.tile_pool(name="opool", bufs=3) as opool,
            tc.tile_pool(name="dram", bufs=1, space="DRAM") as dram,
            tc.tile_pool(name="psum_s", bufs=1, space="PSUM") as psum_s,
            tc.tile_pool(name="psum", bufs=3, space="PSUM") as psum,
        ):
            # ---- loads ----
            exc_s = const.tile([BL, N * 2], I16)
            nc.sync.dma_start(out=exc_s[:], in_=exc[:])
            ne_s = const.tile([BL, 1], F32)
            nc.sync.dma_start(out=ne_s[:], in_=ne[:])
            npr_s = const.tile([BL, 1], F32)
            nc.sync.dma_start(out=npr_s[:], in_=npr[:])
            emb_s = const.tile([26, D], F16)
            nc.gpsimd.dma_start(out=emb_s[:], in_=emb[:])  # casts f32->f16
            iota_s = const.tile([36, 1], F32)
            nc.sync.dma_start(out=iota_s[:], in_=iota_f32[:])

            # ---- per-batch angle columns [BL, 1] ----
            hp = const.tile([BL, 1], F32)
            nc.vector.memset(hp[:], HALF_PI)
            # cos(t) = sin(pi/2 - t) keeps the LUT argument within [-pi, pi]
            ct = const.tile([BL, 1], F32)
            nc.scalar.activation(ct[:], ne_s[:], AF.Sin, bias=hp[:],
                                 scale=-ANGLE_SCALE)
            st = const.tile([BL, 1], F32)
            nc.scalar.activation(st[:], ne_s[:], AF.Sin, bias=0.0, scale=ANGLE_SCALE)
            nst = const.tile([BL, 1], F32)
            nc.scalar.activation(nst[:], ne_s[:], AF.Sin, bias=0.0, scale=-ANGLE_SCALE)
            cp = const.tile([BL, 1], F32)
            nc.scalar.activation(cp[:], npr_s[:], AF.Sin, bias=hp[:],
                                 scale=-ANGLE_SCALE)
            sp = const.tile([BL, 1], F32)
            nc.scalar.activation(sp[:], npr_s[:], AF.Sin, bias=0.0, scale=ANGLE_SCALE)
            nsp = const.tile([BL, 1], F32)
            nc.scalar.activation(nsp[:], npr_s[:], AF.Sin, bias=0.0, scale=-ANGLE_SCALE)

            # natural layout: C_all[b, 4k+i] = (ct,ct,cp,cp)[i],
            #                 S_all[b, 4k+i] = (st,-st,sp,-sp)[i]
            ones = const.tile([BL, 64], F16)
            nc.vector.memset(ones[:], 1.0)
            c_all = const.tile([BL, D], F16)
            s_all = const.tile([BL, D], F16)
            c4 = c_all[:].rearrange("q (k i) -> q k i", i=4)
            s4 = s_all[:].rearrange("q (k i) -> q k i", i=4)
            for i, col in enumerate([ct, ct, cp, cp]):
                nc.vector.tensor_scalar(out=c4[:, :, i], in0=ones[:],
                                        scalar1=col[:], scalar2=None, op0=ALU.mult)
            for i, col in enumerate([st, nst, sp, nsp]):
                nc.vector.tensor_scalar(out=s4[:, :, i], in0=ones[:],
                                        scalar1=col[:], scalar2=None, op0=ALU.mult)

            # ---- select matrix: selT[r, j] = (lut_flat[j] == r), r in [0,26) ----
            lut_bc = const.tile([26, 36], F32)
            nc.sync.dma_start(out=lut_bc[:], in_=lut[0:1, :].to_broadcast((26, 36)))
            selT = const.tile([26, 36], F16)
            nc.vector.tensor_scalar(out=selT[:], in0=lut_bc[:],
                                    scalar1=iota_s[0:26, :], scalar2=None,
                                    op0=ALU.is_equal)

            # ---- 36-row gathered table (natural layout) + pair-swapped copy
            eph_ps = psum_s.tile([36, D], F32)
            nc.tensor.matmul(eph_ps[:], selT[:], emb_s[:], start=True, stop=True)
            e_ph = const.tile([36, D], F16)
            nc.scalar.activation(e_ph[:], eph_ps[:], AF.Copy)
            e_sw = const.tile([36, D], F16)
            ep2 = e_ph[:].rearrange("j (k i) -> j k i", i=2)
            es2 = e_sw[:].rearrange("j (k i) -> j k i", i=2)
            nc.vector.tensor_copy(es2[:, :, 0], ep2[:, :, 1])
            nc.vector.tensor_copy(es2[:, :, 1], ep2[:, :, 0])
            # group-width replicas (4 copies of the table along free dim)
            emb_t4 = const.tile([36, G, D], F16)
            emb_sw4 = const.tile([36, G, D], F16)
            for i in range(G):
                nc.vector.tensor_copy(emb_t4[:, i, :], e_ph[:])
                nc.vector.tensor_copy(emb_sw4[:, i, :], e_sw[:])

            # ---- flat codes: flat[b, n] = 6*a + b (fp16, values < 36) ----
            exc3 = exc_s[:].rearrange("q (n two) -> q n two", two=2)
            a_f = const.tile([BL, N], F32)
            nc.vector.tensor_copy(a_f[:], exc3[:, :, 0])
            b_f = const.tile([BL, N], F32)
            nc.vector.tensor_copy(b_f[:], exc3[:, :, 1])
            flat = const.tile([BL, N], F16)
            nc.vector.scalar_tensor_tensor(out=flat[:], in0=a_f[:], scalar=6.0,
                                           in1=b_f[:], op0=ALU.mult, op1=ALU.add)

            # ---- DRAM bounce, then DRAM->DRAM row staging (36 replicas) ----
            flat_d = dram.tile([BL, N], F16)
            nc.sync.dma_start(out=flat_d[:], in_=flat[:])
            c_all_d = dram.tile([BL, D], F16)
            nc.sync.dma_start(out=c_all_d[:], in_=c_all[:])
            s_all_d = dram.tile([BL, D], F16)
            nc.sync.dma_start(out=s_all_d[:], in_=s_all[:])

            flat_st = dram.tile([36, BL * N], F16)
            nc.sync.dma_start(
                out=flat_st[:],
                in_=flat_d[:].rearrange("q n -> (q n)").unsqueeze(0)
                .to_broadcast((36, BL * N)))
            c_st = dram.tile([36, BL * D], F16)
            nc.sync.dma_start(
                out=c_st[:],
                in_=c_all_d[:].rearrange("q d -> (q d)").unsqueeze(0)
                .to_broadcast((36, BL * D)))
            s_st = dram.tile([36, BL * D], F16)
            nc.sync.dma_start(
                out=s_st[:],
                in_=s_all_d[:].rearrange("q d -> (q d)").unsqueeze(0)
                .to_broadcast((36, BL * D)))

            flat_big = const.tile([36, BL, N], F16)
            rot_big = const.tile([36, BL, D], F16)
            fst3 = flat_st[:].rearrange("j (q n) -> j q n", n=N)
            cst3 = c_st[:].rearrange("j (q d) -> j q d", d=D)
            sst3 = s_st[:].rearrange("j (q d) -> j q d", d=D)

            NG = BL // G
            for g in range(NG):
                gs = slice(g * G, (g + 1) * G)
                # contiguous read-back of the staged replicas for this group
                nc.sync.dma_start(out=flat_big[:, gs, :], in_=fst3[:, gs, :])
                # one-hot in place
                nc.vector.tensor_scalar(out=flat_big[:, gs, :],
                                        in0=flat_big[:, gs, :],
                                        scalar1=iota_s[:], scalar2=None,
                                        op0=ALU.is_equal)
                cbg = gpool.tile([36, G, D], F16, tag="cbg")
                nc.sync.dma_start(out=cbg[:], in_=cst3[:, gs, :])
                sbg = gpool.tile([36, G, D], F16, tag="sbg")
                nc.sync.dma_start(out=sbg[:], in_=sst3[:, gs, :])
                t1g = gpool.tile([36, G, D], F16, tag="t1g")
                nc.vector.tensor_mul(t1g[:], emb_t4[:], cbg[:])
                t2g = gpool.tile([36, G, D], F16, tag="t2g")
                nc.vector.tensor_mul(t2g[:], emb_sw4[:], sbg[:])
                nc.vector.tensor_add(rot_big[:, gs, :], t1g[:], t2g[:])

                for b in range(g * G, (g + 1) * G):
                    ps = psum.tile([128, 4 * D], F32)
                    for c in range(4):
                        # chunk c covers tokens {4k + c}
                        nc.tensor.matmul(ps[:, c * D:(c + 1) * D],
                                         flat_big[:, b, c::4], rot_big[:, b, :],
                                         start=True, stop=True)
                    obuf = opool.tile([128, 4 * D], F32)
                    if b % 4 == 0:
                        nc.vector.tensor_copy(obuf[:], ps[:])
                    else:
                        nc.scalar.activation(obuf[:], ps[:], AF.Copy)
                    # token t = 4k + c sits at obuf[k, c*256:(c+1)*256] ->
                    # this DRAM view is fully linear (contiguous 512 KB write)
                    nc.sync.dma_start(
                        out=out[b].rearrange("(p c) d -> p c d", p=128),
                        in_=obuf[:])

    nc.compile()
    return nc


@functools.lru_cache(maxsize=1)
def _get_nc() -> bass.Bass:
    return build_bass()


def kernel_with_results(excitations, n_electrons, n_protons, emb_weight,
                        lookup_table, trace=False):
    exc = np.asarray(excitations)
    exc16 = exc.astype(np.int16).reshape(B, N * 2)
    ne = np.ascontiguousarray(np.asarray(n_electrons, dtype=np.float32))
    npr = np.ascontiguousarray(np.asarray(n_protons, dtype=np.float32))
    emb = np.ascontiguousarray(np.asarray(emb_weight, dtype=np.float32))
    lut_f = np.asarray(lookup_table).astype(np.float32).reshape(1, 36)
    lut_f = np.ascontiguousarray(lut_f)

    in_maps = []
    for c in range(N_CORES):
        sl = slice(c * BL, (c + 1) * BL)
        in_maps.append({
            "exc": np.ascontiguousarray(exc16[sl]),
            "ne": np.ascontiguousarray(ne[sl].reshape(BL, 1)),
            "npr": np.ascontiguousarray(npr[sl].reshape(BL, 1)),
            "emb": emb,
            "lut": lut_f,
        })

    nc = _get_nc()
    res = run_bass_kernel_spmd(nc, in_maps, list(range(N_CORES)), trace=trace)
    out_arr = np.concatenate(
        [res.results[c]["out"] for c in range(N_CORES)], axis=0)
    return np.ascontiguousarray(out_arr.reshape(B, N, D).astype(np.float32)), res


def kernel(excitations, n_electrons, n_protons, emb_weight, lookup_table):
    out_arr, _ = kernel_with_results(excitations, n_electrons, n_protons,
                                     emb_weight, lookup_table)
    return out_arr


# revision 18
# speedup vs baseline: 1.5540x; 1.1392x over previous
"""Trainium2 Bass kernel for ExcitationEmbedding + Ion RoPE.

Computes, for inputs
  excitations [256, 512, 2] int64 (pairs (a, b) with a, b in [0, 6)),
  n_electrons [256] f32, n_protons [256] f32,
  emb_weight  [26, 256] f32, lookup_table [6, 6] int64:

  idx   = lookup_table[a, b]                       # [B, N]
  emb   = emb_weight[idx]                          # [B, N, D]
  out   = per-batch block-diagonal rotation of emb (theta from n_electrons,
          phi from n_protons, 4-wide blocks: dims (0,1) by theta, (2,3) by phi)

Strategy (pure data parallel over 8 cores, 32 batches each):
  - flat code f = 6*a + b in [0, 36); one-hot against an iota constant.
  - emb36[j] = emb_weight[lut[j]] built once with a select-matmul (invalid
    j rows are 0). Per-batch rotated tables rot[j, b, d] are built with
    plain fp16 tensor ops on group tiles; the per-batch cos/sin patterns
    and flat codes are replicated across the 36 table partitions via
    DRAM->DRAM row staging + contiguous read-back (big DMA packets).
  - Gather is a PE fp16 matmul: chunk c of batch b covers tokens {4k+c};
    all 4 chunks land in one 2-bank PSUM tile, evacuated with a single
    [128, 1024] copy, and the per-batch 512 KB output DMA is fully linear.
  - Work is pipelined in 8 groups of 4 batches.
"""

import functools

import numpy as np

import concourse.bass as bass
import concourse.bacc as bacc
import concourse.mybir as mybir
from concourse import tile
from concourse.bass_utils import run_bass_kernel_spmd

B, N, D = 256, 512, 256
N_CORES = 8
BL = B // N_CORES   # 32 batches per core
G = 4               # batches per pipeline group
ANGLE_SCALE = 0.05
HALF_PI = float(np.pi / 2)

F32 = mybir.dt.float32
F16 = mybir.dt.float16
I16 = mybir.dt.int16
AF = mybir.ActivationFunctionType
ALU = mybir.AluOpType


def build_bass() -> bass.Bass:
    nc = bacc.Bacc(
        "TRN2", target_bir_lowering=False, debug=False, num_devices=N_CORES
    )

    exc = nc.dram_tensor("exc", [BL, N * 2], I16, kind="ExternalInput")
    ne = nc.dram_tensor("ne", [BL, 1], F32, kind="ExternalInput")
    npr = nc.dram_tensor("npr", [BL, 1], F32, kind="ExternalInput")
    emb = nc.dram_tensor("emb", [26, D], F32, kind="ExternalInput")
    lut = nc.dram_tensor("lut", [1, 36], F32, kind="ExternalInput")
    out = nc.dram_tensor("out", [BL, N, D], F32, kind="ExternalOutput")

    iota_f32 = nc.inline_tensor(
        np.arange(36, dtype=np.float32).reshape(36, 1), "iota_f32")

    with tile.TileContext(nc) as tc:
        with (
            tc.tile_pool(name="const", bufs=1) as const,
            tc.tile_pool(name="gpool", bufs=3) as gpool,
            tc.tile_pool(name="opool", bufs=3) as opool,
            tc.tile_pool(name="dram", bufs=1, space="DRAM") as dram,
            tc.tile_pool(name="psum_s", bufs=1, space="PSUM") as psum_s,
            tc.tile_pool(name="psum", bufs=3, space="PSUM") as psum,
        ):
            # ---- loads ----
            exc_s = const.tile([BL, N * 2], I16)
            nc.sync.dma_start(out=exc_s[:], in_=exc[:])
            ne_s = const.tile([BL, 1], F32)
            nc.sync.dma_start(out=ne_s[:], in_=ne[:])
            npr_s = const.tile([BL, 1], F32)
            nc.sync.dma_start(out=npr_s[:], in_=npr[:])
            emb_s = const.tile([26, D], F16)
            nc.gpsimd.dma_start(out=emb_s[:], in_=emb[:])  # casts f32->f16
            iota_s = const.tile([36, 1], F32)
            nc.sync.dma_start(out=iota_s[:], in_=iota_f32[:])

            # ---- per-batch angle columns [BL, 1] ----
            hp = const.tile([BL, 1], F32)
            nc.vector.memset(hp[:], HALF_PI)
            # cos(t) = sin(pi/2 - t) keeps the LUT argument within [-pi, pi]
            ct = const.tile([BL, 1], F32)
            nc.scalar.activation(ct[:], ne_s[:], AF.Sin, bias=hp[:],
                                 scale=-ANGLE_SCALE)
            st = const.tile([BL, 1], F32)
            nc.scalar.activation(st[:], ne_s[:], AF.Sin, bias=0.0, scale=ANGLE_SCALE)
            nst = const.tile([BL, 1], F32)
            nc.scalar.activation(nst[:], ne_s[:], AF.Sin, bias=0.0, scale=-ANGLE_SCALE)
            cp = const.tile([BL, 1], F32)
            nc.scalar.activation(cp[:], npr_s[:], AF.Sin, bias=hp[:],
                                 scale=-ANGLE_SCALE)
            sp = const.tile([BL, 1], F32)
            nc.scalar.activation(sp[:], npr_s[:], AF.Sin, bias=0.0, scale=ANGLE_SCALE)
            nsp = const.tile([BL, 1], F32)
            nc.scalar.activation(nsp[:], npr_s[:], AF.Sin, bias=0.0, scale=-ANGLE_SCALE)

            # natural layout: C_all[b, 4k+i] = (ct,ct,cp,cp)[i],
            #                 S_all[b, 4k+i] = (st,-st,sp,-sp)[i]
            ones = const.tile([BL, 64], F16)
            nc.vector.memset(ones[:], 1.0)
            c_all = const.tile([BL, D], F16)
            s_all = const.tile([BL, D], F16)
            c4 = c_all[:].rearrange("q (k i) -> q k i", i=4)
            s4 = s_all[:].rearrange("q (k i) -> q k i", i=4)
            for i, col in enumerate([ct, ct, cp, cp]):
                nc.vector.tensor_scalar(out=c4[:, :, i], in0=ones[:],
                                        scalar1=col[:], scalar2=None, op0=ALU.mult)
            for i, col in enumerate([st, nst, sp, nsp]):
                nc.vector.tensor_scalar(out=s4[:, :, i], in0=ones[:],
                                        scalar1=col[:], scalar2=None, op0=ALU.mult)

            # ---- select matrix: selT[r, j] = (lut_flat[j] == r), r in [0,26) ----
            lut_bc = const.tile([26, 36], F32)
            nc.sync.dma_start(out=lut_bc[:], in_=lut[0:1, :].to_broadcast((26, 36)))
            selT = const.tile([26, 36], F16)
            nc.vector.tensor_scalar(out=selT[:], in0=lut_bc[:],
                                    scalar1=iota_s[0:26, :], scalar2=None,
                                    op0=ALU.is_equal)

            # ---- 36-row gathered table (natural layout) + pair-swapped copy
            eph_ps = psum_s.tile([36, D], F32)
            nc.tensor.matmul(eph_ps[:], selT[:], emb_s[:], start=True, stop=True)
            e_ph = const.tile([36, D], F16)
            nc.scalar.activation(e_ph[:], eph_ps[:], AF.Copy)
            e_sw = const.tile([36, D], F16)
            ep2 = e_ph[:].rearrange("j (k i) -> j k i", i=2)
            es2 = e_sw[:].rearrange("j (k i) -> j k i", i=2)
            nc.vector.tensor_copy(es2[:, :, 0], ep2[:, :, 1])
            nc.vector.tensor_copy(es2[:, :, 1], ep2[:, :, 0])
            # group-width replicas (4 copies of the table along free dim)
            emb_t4 = const.tile([36, G, D], F16)
            emb_sw4 = const.tile([36, G, D], F16)
            for i in range(G):
                nc.vector.tensor_copy(emb_t4[:, i, :], e_ph[:])
                nc.vector.tensor_copy(emb_sw4[:, i, :], e_sw[:])

            # ---- flat codes: flat[b, n] = 6*a + b (fp16, values < 36) ----
            exc3 = exc_s[:].rearrange("q (n two) -> q n two", two=2)
            a_f = const.tile([BL, N], F32)
            nc.vector.tensor_copy(a_f[:], exc3[:, :, 0])
            b_f = const.tile([BL, N], F32)
            nc.vector.tensor_copy(b_f[:], exc3[:, :, 1])
            flat = const.tile([BL, N], F16)
            nc.vector.scalar_tensor_tensor(out=flat[:], in0=a_f[:], scalar=6.0,
                                           in1=b_f[:], op0=ALU.mult, op1=ALU.add)

            # ---- DRAM bounce; input-side DMAs ride SWDGE so the sync
            # engine's HWDGE FIFO carries only output writes ----
            flat_d = dram.tile([BL, N], F16)
            nc.gpsimd.dma_start(out=flat_d[:], in_=flat[:])
            c_all_d = dram.tile([BL, D], F16)
            nc.gpsimd.dma_start(out=c_all_d[:], in_=c_all[:])
            s_all_d = dram.tile([BL, D], F16)
            nc.gpsimd.dma_start(out=s_all_d[:], in_=s_all[:])

            flat_big = const.tile([36, BL, N], F16)
            rot_big = const.tile([36, BL, D], F16)
            flat_flat = flat_d[:].rearrange("q n -> (q n)")
            c_flat = c_all_d[:].rearrange("q d -> (q d)")
            s_flat = s_all_d[:].rearrange("q d -> (q d)")

            NG = BL // G
            for g in range(NG):
                gs = slice(g * G, (g + 1) * G)
                # broadcast-read of this group's rows: contiguous inner dim
                # -> 36 fat packets per transfer
                nc.gpsimd.dma_start(
                    out=flat_big[:, gs, :],
                    in_=flat_flat[g * G * N:(g + 1) * G * N].unsqueeze(0)
                    .to_broadcast((36, G * N)))
                # one-hot in place
                nc.vector.tensor_scalar(out=flat_big[:, gs, :],
                                        in0=flat_big[:, gs, :],
                                        scalar1=iota_s[:], scalar2=None,
                                        op0=ALU.is_equal)
                cbg = gpool.tile([36, G, D], F16, tag="cbg")
                nc.gpsimd.dma_start(
                    out=cbg[:],
                    in_=c_flat[g * G * D:(g + 1) * G * D].unsqueeze(0)
                    .to_broadcast((36, G * D)))
                sbg = gpool.tile([36, G, D], F16, tag="sbg")
                nc.gpsimd.dma_start(
                    out=sbg[:],
                    in_=s_flat[g * G * D:(g + 1) * G * D].unsqueeze(0)
                    .to_broadcast((36, G * D)))
                t1g = gpool.tile([36, G, D], F16, tag="t1g")
                nc.vector.tensor_mul(t1g[:], emb_t4[:], cbg[:])
                t2g = gpool.tile([36, G, D], F16, tag="t2g")
                nc.vector.tensor_mul(t2g[:], emb_sw4[:], sbg[:])
                nc.vector.tensor_add(rot_big[:, gs, :], t1g[:], t2g[:])

                for b in range(g * G, (g + 1) * G):
                    ps = psum.tile([128, 4 * D], F32)
                    for c in range(4):
                        # chunk c covers tokens {4k + c}
                        nc.tensor.matmul(ps[:, c * D:(c + 1) * D],
                                         flat_big[:, b, c::4], rot_big[:, b, :],
                                         start=True, stop=True)
                    obuf = opool.tile([128, 4 * D], F32)
                    if b % 4 == 0:
                        nc.vector.tensor_copy(obuf[:], ps[:])
                    else:
                        nc.scalar.activation(obuf[:], ps[:], AF.Copy)
                    # token t = 4k + c sits at obuf[k, c*256:(c+1)*256] ->
                    # this DRAM view is fully linear (contiguous 512 KB write)
                    nc.sync.dma_start(
                        out=out[b].rearrange("(p c) d -> p c d", p=128),
                        in_=obuf[:])

    nc.compile()
    return nc


@functools.lru_cache(maxsize=1)
def _get_nc() -> bass.Bass:
    return build_bass()


def kernel_with_results(excitations, n_electrons, n_protons, emb_weight,
                        lookup_table, trace=False):
    exc = np.asarray(excitations)
    exc16 = exc.astype(np.int16).reshape(B, N * 2)
    ne = np.ascontiguousarray(np.asarray(n_electrons, dtype=np.float32))
    npr = np.ascontiguousarray(np.asarray(n_protons, dtype=np.float32))
    emb = np.ascontiguousarray(np.asarray(emb_weight, dtype=np.float32))
    lut_f = np.asarray(lookup_table).astype(np.float32).reshape(1, 36)
    lut_f = np.ascontiguousarray(lut_f)

    in_maps = []
    for c in range(N_CORES):
        sl = slice(c * BL, (c + 1) * BL)
        in_maps.append({
            "exc": np.ascontiguousarray(exc16[sl]),
            "ne": np.ascontiguousarray(ne[sl].reshape(BL, 1)),
            "npr": np.ascontiguousarray(npr[sl].reshape(BL, 1)),
            "emb": emb,
            "lut": lut_f,
        })

    nc = _get_nc()
    res = run_bass_kernel_spmd(nc, in_maps, list(range(N_CORES)), trace=trace)
    out_arr = np.concatenate(
        [res.results[c]["out"] for c in range(N_CORES)], axis=0)
    return np.ascontiguousarray(out_arr.reshape(B, N, D).astype(np.float32)), res


def kernel(excitations, n_electrons, n_protons, emb_weight, lookup_table):
    out_arr, _ = kernel_with_results(excitations, n_electrons, n_protons,
                                     emb_weight, lookup_table)
    return out_arr
